# revision 19
# baseline (speedup 1.0000x reference)
"""Trainium2 Bass kernel for nn_AttentionLayer (B=4, S=2048, H=12, D=64).

Sharding: 8 cores = 4 batches x 2 head-groups (6 heads each).

v5 design (v1 baseline ~371us, v2 ~288us, v4 ~269us):
- Upfront: only the first head-pair's q/k projections + all of V (fat
  3-bank PSUM pool, PE-dense); mc1/mc2 q/k projections run BETWEEN the
  attention pairs, overlapping each pair's ACT pipeline drain, so the
  scalar engine starts exp ~30us earlier.
- Attention scores stay [k_part, q_free] with the two heads of a pair on
  PE row halves (concurrent matmuls); exp on ACT in groups of G=3
  k-chunks per par (PSUM: 2x3 banks scores + 2x1 ctx = 8).
- The ctx matmul is FLIPPED: exp'd probs are stationary (128-col chunks,
  FWL-eligible) and v_aug=[v|ones] moving, so ctx lands [q_part, d] and
  the softmax denominator is per-PARTITION: drain = one [128,260] copy,
  a [128,4] reciprocal_approx_fast, and ONE broadcast tensor_tensor mul.
- ctx accumulates chunk-interleaved into ONE PSUM bank per head with a
  single start=True (first write overwrites: has_written semantics).
- HAM keep-warm: the PE re-throttles to 1.2 GHz unless its activity
  window stays saturated, so each group is padded with dependency-free
  dummy matmuls (v read-only, dead upper half of the ctx banks as the
  target, start=False, never read) that soak up pipeline stalls.
- Boundary-tile 0/1 masks are host-built, DMA'd, applied post-exp, split
  2:1 between DVE and GpSimd.
"""

import sys

if "/opt/trn_rl_repo" not in sys.path:
    sys.path.insert(0, "/opt/trn_rl_repo")

from contextlib import ExitStack

import ml_dtypes
import numpy as np

import concourse.bass as bass
import concourse.mybir as mybir
import concourse.tile as tile
from concourse import bacc
from concourse.bass_utils import run_bass_kernel_spmd

B, S, W, H, D = 4, 2048, 768, 12, 64
NCORES = 8
HPC = 6  # heads per core
QB = 512  # q block (free dim of a scores tile)
KC = 128  # k chunk (partition dim of a scores tile)
NQB = S // QB
NKC = S // KC
NB = 4  # S-chunks for x DMA / projection tiling (512 each)
MC = 3  # 128-row chunks of the 384 per-core W-columns (head pairs)
FKC = W // 128  # feature chunks (contraction for projections)
EW = D + 1  # per-head v_aug width (64 v cols + ones col)
VW = HPC * EW  # v_aug row width per k-chunk
G = 3  # k-chunks exp'd per ACT instruction (3 psum banks)

F32 = mybir.dt.float32
BF16 = mybir.dt.bfloat16

TRACE = False  # set by test.py to profile
LAST_RESULTS = None  # BassKernelResults of the last run (for test.py)


def _ensure_ntff_hook():
    """This image's antenv lacks axon_hooks; register the ctypes NTFF
    profile hook from trn_agent_boot ourselves so trace=True works."""
    import types

    if "antenv.axon_hooks" in sys.modules:
        return
    try:
        from trn_agent_boot.trn_boot import _ntff_profile_via_ctypes

        hook = _ntff_profile_via_ctypes("/opt/axon/libaxon_pjrt.so")
    except Exception:
        hook = None
    mod = types.ModuleType("antenv.axon_hooks")
    mod._hook = hook
    mod.set_axon_ntff_profile_hook = lambda h: setattr(mod, "_hook", h)
    mod.get_axon_ntff_profile_hook = lambda: mod._hook
    sys.modules["antenv.axon_hooks"] = mod
    # artifact upload needs egress this sandbox doesn't have
    import concourse.bass_utils as _bu

    _bu.upload_artifacts = lambda d: "local://" + str(d)


def _classify(seg):
    """Union-over-batches tile classification from segment_ids.

    Returns (cumsums [B,S], per-qb visible k-chunk lists, boundary index).
    Element (k, q) is visible iff cs[k] <= cs[q]; cs is non-decreasing.
    """
    cs = np.cumsum(np.asarray(seg, np.int64), axis=1)
    vis_lists = [[] for _ in range(NQB)]
    bnd_index = {}
    qos = {}  # leading fully-masked 128-q chunks per visible (kc, qb)
    for qb in range(NQB):
        for kc in range(NKC):
            any_computed = False
            all_full_vis = True
            for b in range(B):
                c = cs[b]
                full_mask = c[kc * KC] > c[qb * QB + QB - 1]
                full_vis = c[kc * KC + KC - 1] <= c[qb * QB]
                if not full_mask:
                    any_computed = True
                if not full_vis:
                    all_full_vis = False
            if any_computed:
                vis_lists[qb].append(kc)
                if not all_full_vis:
                    bnd_index[(kc, qb)] = len(bnd_index)
                qo = 0
                for ch in range(4):
                    if all(
                        cs[b][kc * KC] > cs[b][qb * QB + ch * KC + KC - 1]
                        for b in range(B)
                    ):
                        qo += 1
                    else:
                        break
                if qo:
                    qos[(kc, qb)] = qo
    return cs, vis_lists, bnd_index, qos


def _build_program(vis_lists, bnd_index, qos):
    nc = bacc.Bacc()
    n_bnd = max(len(bnd_index), 1)

    xt_d = nc.declare_dram_parameter("xT", [128, NB * FKC * QB], BF16, isOutput=False)
    wq_d = nc.declare_dram_parameter("wq", [128, MC * FKC * 128], BF16, isOutput=False)
    wk_d = nc.declare_dram_parameter("wk", [128, MC * FKC * 128], BF16, isOutput=False)
    wv_d = nc.declare_dram_parameter("wv", [128, FKC * HPC * D], BF16, isOutput=False)
    bqk_d = nc.declare_dram_parameter("bqk", [128, 2 * MC], F32, isOutput=False)
    bvb_d = nc.declare_dram_parameter("bvb", [128, HPC * D], F32, isOutput=False)
    msk_d = nc.declare_dram_parameter("msk", [128, n_bnd * QB], BF16, isOutput=False)
    # output: ctx in [q_part, (qchunk, head, d)] layout, 16 chunks of 128 q
    out_d = nc.declare_dram_parameter("ctx", [128, 4 * NQB * HPC * D], F32, isOutput=True)

    with ExitStack() as ctx:
        tc = ctx.enter_context(tile.TileContext(nc))
        persist = ctx.enter_context(tc.tile_pool(name="persist", bufs=1))

        qt = persist.tile([128, MC * S], BF16)
        kt = persist.tile([128, MC * S], BF16)
        v = persist.tile([128, NKC * VW], BF16)
        ctxq = persist.tile([128, 4 * NQB * HPC * D], F32)
        msk = persist.tile([128, n_bnd * QB], BF16)
        bqk_sb = persist.tile([128, 2 * MC], F32)
        bv_sb = persist.tile([128, HPC * D], F32)
        xt = persist.tile([128, NB * FKC * QB], BF16)
        wq_sb = persist.tile([128, MC * FKC * 128], BF16)
        wk_sb = persist.tile([128, MC * FKC * 128], BF16)
        wv_sb = persist.tile([128, FKC * HPC * D], BF16)

        # DMAs, ordered so the first projection tiles unblock earliest.
        XB = FKC * QB  # xt columns per S-chunk
        WB = FKC * 128  # wq/wk columns per mc chunk
        nc.sync.dma_start(out=wq_sb[:, 0:WB], in_=wq_d[:, 0:WB])
        TB = 2 * QB  # first-chunk piece: 2 feature chunks
        for t in range(3):
            nc.sync.dma_start(out=xt[:, t * TB : (t + 1) * TB], in_=xt_d[:, t * TB : (t + 1) * TB])
        nc.sync.dma_start(out=wk_sb[:, 0:WB], in_=wk_d[:, 0:WB])
        nc.sync.dma_start(out=bqk_sb, in_=bqk_d[:])
        nc.sync.dma_start(out=bv_sb, in_=bvb_d[:])
        nc.sync.dma_start(out=wv_sb, in_=wv_d[:])
        for nb in range(1, NB):
            for t in range(3):
                c0 = nb * XB + t * TB
                nc.sync.dma_start(out=xt[:, c0 : c0 + TB], in_=xt_d[:, c0 : c0 + TB])
        for mc in range(1, MC):
            nc.sync.dma_start(
                out=wq_sb[:, mc * WB : (mc + 1) * WB], in_=wq_d[:, mc * WB : (mc + 1) * WB]
            )
            nc.sync.dma_start(
                out=wk_sb[:, mc * WB : (mc + 1) * WB], in_=wk_d[:, mc * WB : (mc + 1) * WB]
            )
        nc.sync.dma_start(out=msk, in_=msk_d[:])

        # ones columns of v_aug (overwritten nowhere below)
        v_ones = v.rearrange("p (s h e) -> p s h e", h=HPC, e=EW)[:, :, :, D : D + 1]
        nc.vector.memset(v_ones, 1.0)

        def xt_ap(nb, fkc, c0, c1):
            base = nb * XB + fkc * QB
            return xt[:, base + c0 : base + c1]

        def proj_qk(pool, pi, mc, nb):
            w_sb, out_sb = (wq_sb, qt) if pi == 0 else (wk_sb, kt)
            ps = pool.tile([128, QB], F32, tag="fp")
            for fkc in range(FKC):
                nc.tensor.matmul(
                    ps,
                    lhsT=w_sb[:, mc * WB + fkc * 128 : mc * WB + (fkc + 1) * 128],
                    rhs=xt_ap(nb, fkc, 0, QB),
                    start=(fkc == 0),
                    stop=(fkc == FKC - 1),
                )
            nc.vector.tensor_scalar_add(
                out_sb[:, mc * S + nb * QB : mc * S + (nb + 1) * QB],
                ps,
                bqk_sb[:, pi * MC + mc : pi * MC + mc + 1],
            )

        # ---- Phase A: mc0 q/k projections + all of V (PE-dense) ----
        with tc.tile_pool(name="fatproj", bufs=3, space="PSUM") as fat:
            for nb in range(NB):
                proj_qk(fat, 0, 0, nb)
                proj_qk(fat, 1, 0, nb)
            for sc in range(NKC):
                nb, i = sc // 4, sc % 4
                ps = fat.tile([128, HPC * D], F32, tag="fp")
                for fkc in range(FKC):
                    nc.tensor.matmul(
                        ps,
                        lhsT=xt_ap(nb, fkc, i * 128, (i + 1) * 128),
                        rhs=wv_sb[:, fkc * (HPC * D) : (fkc + 1) * (HPC * D)],
                        start=(fkc == 0),
                        stop=(fkc == FKC - 1),
                    )
                dest = v.rearrange("p (s h e) -> p s h e", h=HPC, e=EW)[:, sc, :, 0:D]
                nc.vector.tensor_add(
                    dest,
                    ps.rearrange("p (h e) -> p h e", e=D),
                    bv_sb.rearrange("p (h e) -> p h e", e=D),
                )
            for mc in range(1, MC):
                for nb in range(NB):
                    proj_qk(fat, 1, mc, nb)
                    proj_qk(fat, 0, mc, nb)

        # ---- Phase B: attention; mc1/mc2 projections between pairs ----
        with (
            tc.tile_pool(name="expp", bufs=3) as expp,
            tc.tile_pool(name="drainp", bufs=2) as drainp,
        ):
            mask_rr = 0  # round-robin DVE/GpSimd router for mask muls
            with (
                tc.tile_pool(name="scpsA", bufs=1, space="PSUM") as scps,
                tc.tile_pool(name="ctxpsA", bufs=1, space="PSUM") as ctxps,
            ):
                def emit_ctx(cpk, cesb, ccps, chp, cfirst, is_last):
                    for idx, (kc, joff, qo, w) in enumerate(cpk):
                        last_kc = is_last and (idx == len(cpk) - 1)
                        for par in range(2):
                            hg = 2 * chp + par
                            for chk in range(qo, 4):
                                nc.tensor.matmul(
                                    ccps[par][:, chk * EW : (chk + 1) * EW],
                                    lhsT=cesb[par][:, joff + (chk - qo) * 128 : joff + (chk - qo + 1) * 128],
                                    rhs=v[:, kc * VW + hg * EW : kc * VW + (hg + 1) * EW],
                                    start=(cfirst and chk == 0),
                                    stop=(last_kc and chk == 3),
                                    skip_group_check=True,
                                )
                            if par == 1:
                                cfirst = False
                    return cfirst

                def drain(ccps, chp, cqb):
                    for par in range(2):
                        hg = 2 * chp + par
                        cb = drainp.tile([128, 4 * EW], F32, tag=f"cb{par}", name=f"cb{par}")
                        nc.vector.tensor_copy(cb, ccps[par][:, : 4 * EW])
                        cb_v = cb.rearrange("p (c e) -> p c e", e=EW)
                        rc = drainp.tile([128, 4], F32, tag=f"rc{par}", name=f"rc{par}")
                        nc.vector.tensor_copy(rc, cb_v[:, :, D : D + 1])
                        ri = drainp.tile([128, 4], F32, tag=f"ri{par}", name=f"ri{par}")
                        nc.vector.reciprocal_approx_fast(out=ri, in_=rc)
                        dst = ctxq.rearrange("p (c h e) -> p c h e", h=HPC, e=D)[
                            :, 4 * cqb : 4 * cqb + 4, hg, :
                        ]
                        nc.vector.tensor_tensor(
                            out=dst,
                            in0=cb_v[:, :, 0:D],
                            in1=ri.rearrange("p (c o) -> p c o", o=1).broadcast_to((128, 4, D)),
                            op=mybir.AluOpType.mult,
                        )
                    if chp == MC - 1:
                        cpq = 4 * HPC * D
                        nc.sync.dma_start(
                            out=out_d[:, cqb * cpq : (cqb + 1) * cpq],
                            in_=ctxq[:, cqb * cpq : (cqb + 1) * cpq],
                        )

                # ctx emission is deferred one group GLOBALLY (across qb and
                # pair boundaries) so the next scores always sit ahead of
                # ACT-dependent work in the PE queue. Dummies are suppressed
                # on flush iterations: the fresh ctx banks are still being
                # drained there and the bank tracker would stall the PE.
                pend = None  # (hp, qb, pk, esb, cps, first-holder, last_of_qb)
                for hp in range(MC):
                    for qb in range(NQB):
                        vis = vis_lists[qb]
                        groups = []
                        cur, curw = [], 0
                        for kc in vis:
                            w = QB - qos.get((kc, qb), 0) * KC
                            if curw + w > G * QB and cur:
                                groups.append(cur)
                                cur, curw = [], 0
                            cur.append(kc)
                            curw += w
                        if cur:
                            groups.append(cur)
                        cps = {
                            par: ctxps.tile([128, 512], F32, tag=f"c{par}", name=f"cps{par}")
                            for par in range(2)
                        }
                        qfirst = [True]
                        for gi, g in enumerate(groups):
                            last_g = gi == len(groups) - 1
                            pk = []
                            off = 0
                            for kc in g:
                                qo = qos.get((kc, qb), 0)
                                w = QB - qo * KC
                                pk.append((kc, off, qo, w))
                                off += w
                            n = off
                            sps = {}
                            esb = {}
                            for par in range(2):
                                sps[par] = scps.tile(
                                    [128, G * QB], F32, tag=f"s{par}", name=f"sps{par}"
                                )
                                esb[par] = expp.tile(
                                    [128, G * QB], BF16, tag=f"e{par}", name=f"esb{par}"
                                )
                            for kc, joff, qo, w in pk:
                                for par in range(2):
                                    po = par * 64
                                    nc.tensor.matmul(
                                        sps[par][:, joff : joff + w],
                                        lhsT=kt[po : po + 64, hp * S + kc * KC : hp * S + kc * KC + KC],
                                        rhs=qt[po : po + 64, hp * S + qb * QB + qo * KC : hp * S + (qb + 1) * QB],
                                        start=True,
                                        stop=True,
                                    )
                            for par in range(2):
                                nc.scalar.activation(
                                    out=esb[par][:, :n],
                                    in_=sps[par][:, :n],
                                    func=mybir.ActivationFunctionType.Exp,
                                    scale=0.125,
                                )
                            for kc, joff, qo, w in pk:
                                bi = bnd_index.get((kc, qb))
                                if bi is not None:
                                    for par in range(2):
                                        eng = nc.gpsimd if mask_rr % 3 == 0 else nc.vector
                                        mask_rr += 1
                                        eng.tensor_mul(
                                            esb[par][:, joff : joff + w],
                                            esb[par][:, joff : joff + w],
                                            msk[:, bi * QB + qo * KC : (bi + 1) * QB],
                                        )
                            drained = False
                            if pend is not None:
                                php, pqb, ppk, pesb, pcps, pf, plast = pend
                                pf[0] = emit_ctx(ppk, pesb, pcps, php, pf[0], plast)
                                if plast:
                                    drain(pcps, php, pqb)
                                    drained = True
                            pend = (hp, qb, pk, esb, cps, qfirst, last_g)
                            nd = 0 if drained else (6 if not last_g else 3)
                            for di in range(nd):
                                for par in range(2):
                                    nc.tensor.matmul(
                                        cps[par][:, 260:512],
                                        lhsT=v[:, 0:128],
                                        rhs=v[:, 0:252],
                                        start=False,
                                        stop=False,
                                        skip_group_check=True,
                                    )
                php, pqb, ppk, pesb, pcps, pf, plast = pend
                emit_ctx(ppk, pesb, pcps, php, pf[0], True)
                drain(pcps, php, pqb)

    nc.finalize()
    return nc


def _core_inputs(x, segment_ids, Wq, bq, Wk, bk, Wv, bv, cs, bnd_index, core):
    b, h0 = core // 2, HPC * (core % 2)
    cols = slice(h0 * D, (h0 + HPC) * D)
    xT = np.ascontiguousarray(x[b].T)  # [768, 2048]
    # [128, nb, fkc, 512] layout: S-major chunks, feature-chunk minor
    xt_s = (
        xT.reshape(FKC, 128, NB, QB).transpose(1, 2, 0, 3).reshape(128, NB * FKC * QB)
    ).astype(ml_dtypes.bfloat16)

    def wqk_prep(Wm):
        ws = Wm[:, cols]  # [768, 384]
        return np.ascontiguousarray(
            ws.reshape(FKC, 128, MC, 128).transpose(1, 2, 0, 3).reshape(128, MC * FKC * 128)
        ).astype(ml_dtypes.bfloat16)

    ws = Wv[:, cols]
    wv_s = np.ascontiguousarray(
        ws.reshape(FKC, 128, HPC * D).transpose(1, 0, 2).reshape(128, FKC * HPC * D)
    ).astype(ml_dtypes.bfloat16)

    bq_s = np.ascontiguousarray(bq[cols].reshape(MC, 128).T)
    bk_s = np.ascontiguousarray(bk[cols].reshape(MC, 128).T)
    bqk = np.concatenate([bq_s, bk_s], axis=1)  # [128, 6]
    bvb = np.ascontiguousarray(np.broadcast_to(bv[cols], (128, HPC * D)))

    csb = cs[b]
    n_bnd = max(len(bnd_index), 1)
    mskv = np.zeros((128, n_bnd * QB), np.float32)
    for (kc, qb), bi in bnd_index.items():
        mskv[:, bi * QB : (bi + 1) * QB] = (
            csb[kc * KC : (kc + 1) * KC, None] <= csb[None, qb * QB : (qb + 1) * QB]
        )
    return {
        "xT": np.ascontiguousarray(xt_s),
        "wq": wqk_prep(Wq),
        "wk": wqk_prep(Wk),
        "wv": wv_s,
        "bqk": np.ascontiguousarray(bqk),
        "bvb": bvb,
        "msk": mskv.astype(ml_dtypes.bfloat16),
    }


def kernel(x, segment_ids, Wq, bq, Wk, bk, Wv, bv):
    global LAST_RESULTS
    x = np.asarray(x, np.float32)
    segment_ids = np.asarray(segment_ids)
    Wq, bq = np.asarray(Wq, np.float32), np.asarray(bq, np.float32)
    Wk, bk = np.asarray(Wk, np.float32), np.asarray(bk, np.float32)
    Wv, bv = np.asarray(Wv, np.float32), np.asarray(bv, np.float32)

    cs, vis_lists, bnd_index, qos = _classify(segment_ids)
    nc = _build_program(vis_lists, bnd_index, qos)
    in_maps = [
        _core_inputs(x, segment_ids, Wq, bq, Wk, bk, Wv, bv, cs, bnd_index, c)
        for c in range(NCORES)
    ]
    if TRACE:
        _ensure_ntff_hook()
    res = run_bass_kernel_spmd(nc, in_maps, list(range(NCORES)), trace=TRACE)
    LAST_RESULTS = res

    out = np.empty((B, S, W), np.float32)
    for c in range(NCORES):
        b, h0 = c // 2, HPC * (c % 2)
        # [128, 16, 384] -> [16, 128, 384] -> [2048, 384]
        cq = res.results[c]["ctx"].reshape(128, 4 * NQB, HPC * D)
        out[b, :, h0 * D : (h0 + HPC) * D] = cq.transpose(1, 0, 2).reshape(S, HPC * D)
    return out


# revision 21
# speedup vs baseline: 1.0042x; 1.0042x over previous
"""Trainium2 Bass kernel for nn_AttentionLayer (B=4, S=2048, H=12, D=64).

Sharding: 8 cores = 4 batches x 2 head-groups (6 heads each).

v5 design (v1 baseline ~371us, v2 ~288us, v4 ~269us):
- Upfront: only the first head-pair's q/k projections + all of V (fat
  3-bank PSUM pool, PE-dense); mc1/mc2 q/k projections run BETWEEN the
  attention pairs, overlapping each pair's ACT pipeline drain, so the
  scalar engine starts exp ~30us earlier.
- Attention scores stay [k_part, q_free] with the two heads of a pair on
  PE row halves (concurrent matmuls); exp on ACT in groups of G=3
  k-chunks per par (PSUM: 2x3 banks scores + 2x1 ctx = 8).
- The ctx matmul is FLIPPED: exp'd probs are stationary (128-col chunks,
  FWL-eligible) and v_aug=[v|ones] moving, so ctx lands [q_part, d] and
  the softmax denominator is per-PARTITION: drain = one [128,260] copy,
  a [128,4] reciprocal_approx_fast, and ONE broadcast tensor_tensor mul.
- ctx accumulates chunk-interleaved into ONE PSUM bank per head with a
  single start=True (first write overwrites: has_written semantics).
- HAM keep-warm: the PE re-throttles to 1.2 GHz unless its activity
  window stays saturated, so each group is padded with dependency-free
  dummy matmuls (v read-only, dead upper half of the ctx banks as the
  target, start=False, never read) that soak up pipeline stalls.
- Boundary-tile 0/1 masks are host-built, DMA'd, applied post-exp, split
  2:1 between DVE and GpSimd.
"""

import sys

if "/opt/trn_rl_repo" not in sys.path:
    sys.path.insert(0, "/opt/trn_rl_repo")

from contextlib import ExitStack

import ml_dtypes
import numpy as np

import concourse.bass as bass
import concourse.mybir as mybir
import concourse.tile as tile
from concourse import bacc
from concourse.bass_utils import run_bass_kernel_spmd

B, S, W, H, D = 4, 2048, 768, 12, 64
NCORES = 8
HPC = 6  # heads per core
QB = 512  # q block (free dim of a scores tile)
KC = 128  # k chunk (partition dim of a scores tile)
NQB = S // QB
NKC = S // KC
NB = 4  # S-chunks for x DMA / projection tiling (512 each)
MC = 3  # 128-row chunks of the 384 per-core W-columns (head pairs)
FKC = W // 128  # feature chunks (contraction for projections)
EW = D + 1  # per-head v_aug width (64 v cols + ones col)
VW = HPC * EW  # v_aug row width per k-chunk
G = 3  # k-chunks exp'd per ACT instruction (3 psum banks)

F32 = mybir.dt.float32
BF16 = mybir.dt.bfloat16

TRACE = False  # set by test.py to profile
LAST_RESULTS = None  # BassKernelResults of the last run (for test.py)


def _ensure_ntff_hook():
    """This image's antenv lacks axon_hooks; register the ctypes NTFF
    profile hook from trn_agent_boot ourselves so trace=True works."""
    import types

    if "antenv.axon_hooks" in sys.modules:
        return
    try:
        from trn_agent_boot.trn_boot import _ntff_profile_via_ctypes

        hook = _ntff_profile_via_ctypes("/opt/axon/libaxon_pjrt.so")
    except Exception:
        hook = None
    mod = types.ModuleType("antenv.axon_hooks")
    mod._hook = hook
    mod.set_axon_ntff_profile_hook = lambda h: setattr(mod, "_hook", h)
    mod.get_axon_ntff_profile_hook = lambda: mod._hook
    sys.modules["antenv.axon_hooks"] = mod
    # artifact upload needs egress this sandbox doesn't have
    import concourse.bass_utils as _bu

    _bu.upload_artifacts = lambda d: "local://" + str(d)


def _classify(seg):
    """Union-over-batches tile classification from segment_ids.

    Returns (cumsums [B,S], per-qb visible k-chunk lists, boundary index).
    Element (k, q) is visible iff cs[k] <= cs[q]; cs is non-decreasing.
    """
    cs = np.cumsum(np.asarray(seg, np.int64), axis=1)
    vis_lists = [[] for _ in range(NQB)]
    bnd_index = {}
    qos = {}  # leading fully-masked 128-q chunks per visible (kc, qb)
    for qb in range(NQB):
        for kc in range(NKC):
            any_computed = False
            all_full_vis = True
            for b in range(B):
                c = cs[b]
                full_mask = c[kc * KC] > c[qb * QB + QB - 1]
                full_vis = c[kc * KC + KC - 1] <= c[qb * QB]
                if not full_mask:
                    any_computed = True
                if not full_vis:
                    all_full_vis = False
            if any_computed:
                vis_lists[qb].append(kc)
                if not all_full_vis:
                    bnd_index[(kc, qb)] = len(bnd_index)
                qo = 0
                for ch in range(4):
                    if all(
                        cs[b][kc * KC] > cs[b][qb * QB + ch * KC + KC - 1]
                        for b in range(B)
                    ):
                        qo += 1
                    else:
                        break
                if qo:
                    qos[(kc, qb)] = qo
    return cs, vis_lists, bnd_index, qos


def _build_program(vis_lists, bnd_index, qos):
    nc = bacc.Bacc()
    n_bnd = max(len(bnd_index), 1)

    xt_d = nc.declare_dram_parameter("xT", [128, NB * FKC * QB], BF16, isOutput=False)
    wq_d = nc.declare_dram_parameter("wq", [128, MC * FKC * 128], BF16, isOutput=False)
    wk_d = nc.declare_dram_parameter("wk", [128, MC * FKC * 128], BF16, isOutput=False)
    wv_d = nc.declare_dram_parameter("wv", [128, FKC * HPC * D], BF16, isOutput=False)
    bqk_d = nc.declare_dram_parameter("bqk", [128, 2 * MC], F32, isOutput=False)
    bvb_d = nc.declare_dram_parameter("bvb", [128, HPC * D], F32, isOutput=False)
    msk_d = nc.declare_dram_parameter("msk", [128, n_bnd * QB], BF16, isOutput=False)
    # output: ctx in [q_part, (qchunk, head, d)] layout, 16 chunks of 128 q
    out_d = nc.declare_dram_parameter("ctx", [128, 4 * NQB * HPC * D], F32, isOutput=True)

    with ExitStack() as ctx:
        tc = ctx.enter_context(tile.TileContext(nc))
        persist = ctx.enter_context(tc.tile_pool(name="persist", bufs=1))

        qt = persist.tile([128, MC * S], BF16)
        kt = persist.tile([128, MC * S], BF16)
        v = persist.tile([128, NKC * VW], BF16)
        ctxq = persist.tile([128, 4 * NQB * HPC * D], F32)
        msk = persist.tile([128, n_bnd * QB], BF16)
        bqk_sb = persist.tile([128, 2 * MC], F32)
        bv_sb = persist.tile([128, HPC * D], F32)
        xt = persist.tile([128, NB * FKC * QB], BF16)
        wq_sb = persist.tile([128, MC * FKC * 128], BF16)
        wk_sb = persist.tile([128, MC * FKC * 128], BF16)
        wv_sb = persist.tile([128, FKC * HPC * D], BF16)

        # DMAs, ordered so the first projection tiles unblock earliest.
        XB = FKC * QB  # xt columns per S-chunk
        WB = FKC * 128  # wq/wk columns per mc chunk
        nc.sync.dma_start(out=wq_sb[:, 0:WB], in_=wq_d[:, 0:WB])
        TB = 2 * QB  # first-chunk piece: 2 feature chunks
        for t in range(3):
            nc.sync.dma_start(out=xt[:, t * TB : (t + 1) * TB], in_=xt_d[:, t * TB : (t + 1) * TB])
        nc.sync.dma_start(out=wk_sb[:, 0:WB], in_=wk_d[:, 0:WB])
        nc.sync.dma_start(out=bqk_sb, in_=bqk_d[:])
        nc.sync.dma_start(out=bv_sb, in_=bvb_d[:])
        nc.sync.dma_start(out=wv_sb, in_=wv_d[:])
        for nb in range(1, NB):
            for t in range(3):
                c0 = nb * XB + t * TB
                nc.sync.dma_start(out=xt[:, c0 : c0 + TB], in_=xt_d[:, c0 : c0 + TB])
        for mc in range(1, MC):
            nc.sync.dma_start(
                out=wq_sb[:, mc * WB : (mc + 1) * WB], in_=wq_d[:, mc * WB : (mc + 1) * WB]
            )
            nc.sync.dma_start(
                out=wk_sb[:, mc * WB : (mc + 1) * WB], in_=wk_d[:, mc * WB : (mc + 1) * WB]
            )
        nc.sync.dma_start(out=msk, in_=msk_d[:])

        # ones columns of v_aug (overwritten nowhere below)
        v_ones = v.rearrange("p (s h e) -> p s h e", h=HPC, e=EW)[:, :, :, D : D + 1]
        nc.vector.memset(v_ones, 1.0)

        def xt_ap(nb, fkc, c0, c1):
            base = nb * XB + fkc * QB
            return xt[:, base + c0 : base + c1]

        def proj_qk(pool, pi, mc, nb):
            w_sb, out_sb = (wq_sb, qt) if pi == 0 else (wk_sb, kt)
            ps = pool.tile([128, QB], F32, tag="fp")
            for fkc in range(FKC):
                nc.tensor.matmul(
                    ps,
                    lhsT=w_sb[:, mc * WB + fkc * 128 : mc * WB + (fkc + 1) * 128],
                    rhs=xt_ap(nb, fkc, 0, QB),
                    start=(fkc == 0),
                    stop=(fkc == FKC - 1),
                )
            nc.vector.tensor_scalar_add(
                out_sb[:, mc * S + nb * QB : mc * S + (nb + 1) * QB],
                ps,
                bqk_sb[:, pi * MC + mc : pi * MC + mc + 1],
            )

        # ---- Phase A: mc0 q/k projections + all of V (PE-dense) ----
        with tc.tile_pool(name="fatproj", bufs=3, space="PSUM") as fat:

            def proj_v(sc):
                nb, i = sc // 4, sc % 4
                ps = fat.tile([128, HPC * D], F32, tag="fp")
                for fkc in range(FKC):
                    nc.tensor.matmul(
                        ps,
                        lhsT=xt_ap(nb, fkc, i * 128, (i + 1) * 128),
                        rhs=wv_sb[:, fkc * (HPC * D) : (fkc + 1) * (HPC * D)],
                        start=(fkc == 0),
                        stop=(fkc == FKC - 1),
                    )
                dest = v.rearrange("p (s h e) -> p s h e", h=HPC, e=EW)[:, sc, :, 0:D]
                nc.vector.tensor_add(
                    dest,
                    ps.rearrange("p (h e) -> p h e", e=D),
                    bv_sb.rearrange("p (h e) -> p h e", e=D),
                )

            # consume tiles in DMA-arrival order: qk nb0-1, v sc0-7 (xt0-1 +
            # wv), then qk nb2-3 and v sc8-15 once xt2-3 have landed
            for nb in range(2):
                proj_qk(fat, 0, 0, nb)
                proj_qk(fat, 1, 0, nb)
            for sc in range(8):
                proj_v(sc)
            for nb in range(2, NB):
                proj_qk(fat, 0, 0, nb)
                proj_qk(fat, 1, 0, nb)
            for sc in range(8, NKC):
                proj_v(sc)
            for mc in range(1, MC):
                for nb in range(NB):
                    proj_qk(fat, 1, mc, nb)
                    proj_qk(fat, 0, mc, nb)

        # ---- Phase B: attention; mc1/mc2 projections between pairs ----
        with (
            tc.tile_pool(name="expp", bufs=3) as expp,
            tc.tile_pool(name="drainp", bufs=2) as drainp,
        ):
            mask_rr = 0  # round-robin DVE/GpSimd router for mask muls
            with (
                tc.tile_pool(name="scpsA", bufs=1, space="PSUM") as scps,
                tc.tile_pool(name="ctxpsA", bufs=1, space="PSUM") as ctxps,
            ):
                for hp in range(MC):
                    for qb in range(NQB):
                        vis = vis_lists[qb]
                        groups = []
                        cur, curw = [], 0
                        for kc in vis:
                            w = QB - qos.get((kc, qb), 0) * KC
                            if curw + w > G * QB and cur:
                                groups.append(cur)
                                cur, curw = [], 0
                            cur.append(kc)
                            curw += w
                        if cur:
                            groups.append(cur)
                        cps = {
                            par: ctxps.tile([128, 512], F32, tag=f"c{par}", name=f"cps{par}")
                            for par in range(2)
                        }
                        first = True
                        n_done = 0
                        deferred = None  # (g, esb) of the previous group
                        for g in groups:
                            # pack only the visible q-subrange of each kc
                            pk = []
                            off = 0
                            for kc in g:
                                qo = qos.get((kc, qb), 0)
                                w = QB - qo * KC
                                pk.append((kc, off, qo, w))
                                off += w
                            n = off
                            sps = {}
                            esb = {}
                            for par in range(2):
                                sps[par] = scps.tile(
                                    [128, G * QB], F32, tag=f"s{par}", name=f"sps{par}"
                                )
                                esb[par] = expp.tile(
                                    [128, G * QB], BF16, tag=f"e{par}", name=f"esb{par}"
                                )
                            for kc, joff, qo, w in pk:
                                for par in range(2):
                                    po = par * 64
                                    nc.tensor.matmul(
                                        sps[par][:, joff : joff + w],
                                        lhsT=kt[po : po + 64, hp * S + kc * KC : hp * S + kc * KC + KC],
                                        rhs=qt[po : po + 64, hp * S + qb * QB + qo * KC : hp * S + (qb + 1) * QB],
                                        start=True,
                                        stop=True,
                                    )
                            for par in range(2):
                                nc.scalar.activation(
                                    out=esb[par][:, :n],
                                    in_=sps[par][:, :n],
                                    func=mybir.ActivationFunctionType.Exp,
                                    scale=0.125,
                                )
                            # boundary masks (0/1, post-exp), 2:1 DVE:GpSimd
                            for kc, joff, qo, w in pk:
                                bi = bnd_index.get((kc, qb))
                                if bi is not None:
                                    for par in range(2):
                                        eng = nc.gpsimd if mask_rr % 3 == 0 else nc.vector
                                        mask_rr += 1
                                        eng.tensor_mul(
                                            esb[par][:, joff : joff + w],
                                            esb[par][:, joff : joff + w],
                                            msk[:, bi * QB + qo * KC : (bi + 1) * QB],
                                        )
                            # flipped ctx (esb stationary, v_aug moving) is
                            # emitted ONE GROUP LATE so the next group's
                            # scores matmuls sit ahead of ACT-dependent work
                            # in the PE queue (else ACT gates its own input).
                            n_done += len(g)
                            last_g = n_done == len(vis)

                            def emit_ctx(cpk, cesb, is_last):
                                nonlocal first
                                for idx, (kc, joff, qo, w) in enumerate(cpk):
                                    last_kc = is_last and (idx == len(cpk) - 1)
                                    for par in range(2):
                                        hg = 2 * hp + par
                                        for chk in range(qo, 4):
                                            nc.tensor.matmul(
                                                cps[par][:, chk * EW : (chk + 1) * EW],
                                                lhsT=cesb[par][:, joff + (chk - qo) * 128 : joff + (chk - qo + 1) * 128],
                                                rhs=v[:, kc * VW + hg * EW : kc * VW + (hg + 1) * EW],
                                                start=(first and chk == 0),
                                                stop=(last_kc and chk == 3),
                                                skip_group_check=True,
                                            )
                                        if par == 1:
                                            first = False

                            if deferred is not None:
                                emit_ctx(deferred[0], deferred[1], False)
                            deferred = (pk, esb)
                            # PE keep-warm: the HAM re-throttles the PE to
                            # 1.2 GHz unless its activity window stays
                            # saturated, so pad each group with dependency-
                            # free dummy matmuls (v read-only, dead upper
                            # half of the ctx banks, start=False, never read).
                            nd = 6 if not last_g else 3
                            for di in range(nd):
                                for par in range(2):
                                    nc.tensor.matmul(
                                        cps[par][:, 260:512],
                                        lhsT=v[:, 0:128],
                                        rhs=v[:, 0:252],
                                        start=False,
                                        stop=False,
                                        skip_group_check=True,
                                    )
                        emit_ctx(deferred[0], deferred[1], True)
                        # drain: per head, copy bank out, recip the 4 denom
                        # cols, one broadcast mul into the [q,d] output tile
                        for par in range(2):
                            hg = 2 * hp + par
                            cb = drainp.tile([128, 4 * EW], F32, tag=f"cb{par}", name=f"cb{par}")
                            nc.vector.tensor_copy(cb, cps[par][:, : 4 * EW])
                            cb_v = cb.rearrange("p (c e) -> p c e", e=EW)
                            rc = drainp.tile([128, 4], F32, tag=f"rc{par}", name=f"rc{par}")
                            nc.vector.tensor_copy(rc, cb_v[:, :, D : D + 1])
                            ri = drainp.tile([128, 4], F32, tag=f"ri{par}", name=f"ri{par}")
                            nc.vector.reciprocal_approx_fast(out=ri, in_=rc)
                            dst = ctxq.rearrange("p (c h e) -> p c h e", h=HPC, e=D)[
                                :, 4 * qb : 4 * qb + 4, hg, :
                            ]
                            nc.vector.tensor_tensor(
                                out=dst,
                                in0=cb_v[:, :, 0:D],
                                in1=ri.rearrange("p (c o) -> p c o", o=1).broadcast_to((128, 4, D)),
                                op=mybir.AluOpType.mult,
                            )
                        if hp == MC - 1:
                            cpq = 4 * HPC * D  # output columns per qb
                            nc.sync.dma_start(
                                out=out_d[:, qb * cpq : (qb + 1) * cpq],
                                in_=ctxq[:, qb * cpq : (qb + 1) * cpq],
                            )

    nc.finalize()
    return nc


def _core_inputs(x, segment_ids, Wq, bq, Wk, bk, Wv, bv, cs, bnd_index, core):
    b, h0 = core // 2, HPC * (core % 2)
    cols = slice(h0 * D, (h0 + HPC) * D)
    xT = np.ascontiguousarray(x[b].T)  # [768, 2048]
    # [128, nb, fkc, 512] layout: S-major chunks, feature-chunk minor
    xt_s = (
        xT.reshape(FKC, 128, NB, QB).transpose(1, 2, 0, 3).reshape(128, NB * FKC * QB)
    ).astype(ml_dtypes.bfloat16)

    def wqk_prep(Wm):
        ws = Wm[:, cols]  # [768, 384]
        return np.ascontiguousarray(
            ws.reshape(FKC, 128, MC, 128).transpose(1, 2, 0, 3).reshape(128, MC * FKC * 128)
        ).astype(ml_dtypes.bfloat16)

    ws = Wv[:, cols]
    wv_s = np.ascontiguousarray(
        ws.reshape(FKC, 128, HPC * D).transpose(1, 0, 2).reshape(128, FKC * HPC * D)
    ).astype(ml_dtypes.bfloat16)

    bq_s = np.ascontiguousarray(bq[cols].reshape(MC, 128).T)
    bk_s = np.ascontiguousarray(bk[cols].reshape(MC, 128).T)
    bqk = np.concatenate([bq_s, bk_s], axis=1)  # [128, 6]
    bvb = np.ascontiguousarray(np.broadcast_to(bv[cols], (128, HPC * D)))

    csb = cs[b]
    n_bnd = max(len(bnd_index), 1)
    mskv = np.zeros((128, n_bnd * QB), np.float32)
    for (kc, qb), bi in bnd_index.items():
        mskv[:, bi * QB : (bi + 1) * QB] = (
            csb[kc * KC : (kc + 1) * KC, None] <= csb[None, qb * QB : (qb + 1) * QB]
        )
    return {
        "xT": np.ascontiguousarray(xt_s),
        "wq": wqk_prep(Wq),
        "wk": wqk_prep(Wk),
        "wv": wv_s,
        "bqk": np.ascontiguousarray(bqk),
        "bvb": bvb,
        "msk": mskv.astype(ml_dtypes.bfloat16),
    }


def kernel(x, segment_ids, Wq, bq, Wk, bk, Wv, bv):
    global LAST_RESULTS
    x = np.asarray(x, np.float32)
    segment_ids = np.asarray(segment_ids)
    Wq, bq = np.asarray(Wq, np.float32), np.asarray(bq, np.float32)
    Wk, bk = np.asarray(Wk, np.float32), np.asarray(bk, np.float32)
    Wv, bv = np.asarray(Wv, np.float32), np.asarray(bv, np.float32)

    cs, vis_lists, bnd_index, qos = _classify(segment_ids)
    nc = _build_program(vis_lists, bnd_index, qos)
    in_maps = [
        _core_inputs(x, segment_ids, Wq, bq, Wk, bk, Wv, bv, cs, bnd_index, c)
        for c in range(NCORES)
    ]
    if TRACE:
        _ensure_ntff_hook()
    res = run_bass_kernel_spmd(nc, in_maps, list(range(NCORES)), trace=TRACE)
    LAST_RESULTS = res

    out = np.empty((B, S, W), np.float32)
    for c in range(NCORES):
        b, h0 = c // 2, HPC * (c % 2)
        # [128, 16, 384] -> [16, 128, 384] -> [2048, 384]
        cq = res.results[c]["ctx"].reshape(128, 4 * NQB, HPC * D)
        out[b, :, h0 * D : (h0 + HPC) * D] = cq.transpose(1, 0, 2).reshape(S, HPC * D)
    return out


# revision 23
# speedup vs baseline: 1.0165x; 1.0123x over previous
"""Trainium2 Bass kernel for nn_AttentionLayer (B=4, S=2048, H=12, D=64).

Sharding: 8 cores = 4 batches x 2 head-groups (6 heads each).

v5 design (v1 baseline ~371us, v2 ~288us, v4 ~269us):
- Upfront: only the first head-pair's q/k projections + all of V (fat
  3-bank PSUM pool, PE-dense); mc1/mc2 q/k projections run BETWEEN the
  attention pairs, overlapping each pair's ACT pipeline drain, so the
  scalar engine starts exp ~30us earlier.
- Attention scores stay [k_part, q_free] with the two heads of a pair on
  PE row halves (concurrent matmuls); exp on ACT in groups of G=3
  k-chunks per par (PSUM: 2x3 banks scores + 2x1 ctx = 8).
- The ctx matmul is FLIPPED: exp'd probs are stationary (128-col chunks,
  FWL-eligible) and v_aug=[v|ones] moving, so ctx lands [q_part, d] and
  the softmax denominator is per-PARTITION: drain = one [128,260] copy,
  a [128,4] reciprocal_approx_fast, and ONE broadcast tensor_tensor mul.
- ctx accumulates chunk-interleaved into ONE PSUM bank per head with a
  single start=True (first write overwrites: has_written semantics).
- HAM keep-warm: the PE re-throttles to 1.2 GHz unless its activity
  window stays saturated, so each group is padded with dependency-free
  dummy matmuls (v read-only, dead upper half of the ctx banks as the
  target, start=False, never read) that soak up pipeline stalls.
- Boundary-tile 0/1 masks are host-built, DMA'd, applied post-exp, split
  2:1 between DVE and GpSimd.
"""

import sys

if "/opt/trn_rl_repo" not in sys.path:
    sys.path.insert(0, "/opt/trn_rl_repo")

from contextlib import ExitStack

import ml_dtypes
import numpy as np

import concourse.bass as bass
import concourse.mybir as mybir
import concourse.tile as tile
from concourse import bacc
from concourse.bass_utils import run_bass_kernel_spmd

B, S, W, H, D = 4, 2048, 768, 12, 64
NCORES = 8
HPC = 6  # heads per core
QB = 512  # q block (free dim of a scores tile)
KC = 128  # k chunk (partition dim of a scores tile)
NQB = S // QB
NKC = S // KC
NB = 4  # S-chunks for x DMA / projection tiling (512 each)
MC = 3  # 128-row chunks of the 384 per-core W-columns (head pairs)
FKC = W // 128  # feature chunks (contraction for projections)
EW = D + 1  # per-head v_aug width (64 v cols + ones col)
VW = HPC * EW  # v_aug row width per k-chunk
G = 3  # k-chunks exp'd per ACT instruction (3 psum banks)

F32 = mybir.dt.float32
BF16 = mybir.dt.bfloat16

TRACE = False  # set by test.py to profile
LAST_RESULTS = None  # BassKernelResults of the last run (for test.py)


def _ensure_ntff_hook():
    """This image's antenv lacks axon_hooks; register the ctypes NTFF
    profile hook from trn_agent_boot ourselves so trace=True works."""
    import types

    if "antenv.axon_hooks" in sys.modules:
        return
    try:
        from trn_agent_boot.trn_boot import _ntff_profile_via_ctypes

        hook = _ntff_profile_via_ctypes("/opt/axon/libaxon_pjrt.so")
    except Exception:
        hook = None
    mod = types.ModuleType("antenv.axon_hooks")
    mod._hook = hook
    mod.set_axon_ntff_profile_hook = lambda h: setattr(mod, "_hook", h)
    mod.get_axon_ntff_profile_hook = lambda: mod._hook
    sys.modules["antenv.axon_hooks"] = mod
    # artifact upload needs egress this sandbox doesn't have
    import concourse.bass_utils as _bu

    _bu.upload_artifacts = lambda d: "local://" + str(d)


def _classify(seg):
    """Union-over-batches tile classification from segment_ids.

    Returns (cumsums [B,S], per-qb visible k-chunk lists, boundary index).
    Element (k, q) is visible iff cs[k] <= cs[q]; cs is non-decreasing.
    """
    cs = np.cumsum(np.asarray(seg, np.int64), axis=1)
    vis_lists = [[] for _ in range(NQB)]
    bnd_index = {}
    qos = {}  # leading fully-masked 128-q chunks per visible (kc, qb)
    for qb in range(NQB):
        for kc in range(NKC):
            any_computed = False
            all_full_vis = True
            for b in range(B):
                c = cs[b]
                full_mask = c[kc * KC] > c[qb * QB + QB - 1]
                full_vis = c[kc * KC + KC - 1] <= c[qb * QB]
                if not full_mask:
                    any_computed = True
                if not full_vis:
                    all_full_vis = False
            if any_computed:
                vis_lists[qb].append(kc)
                if not all_full_vis:
                    bnd_index[(kc, qb)] = len(bnd_index)
                qo = 0
                for ch in range(4):
                    if all(
                        cs[b][kc * KC] > cs[b][qb * QB + ch * KC + KC - 1]
                        for b in range(B)
                    ):
                        qo += 1
                    else:
                        break
                if qo:
                    qos[(kc, qb)] = qo
    return cs, vis_lists, bnd_index, qos


def _build_program(vis_lists, bnd_index, qos):
    nc = bacc.Bacc()
    n_bnd = max(len(bnd_index), 1)

    xt_d = nc.declare_dram_parameter("xT", [128, NB * FKC * QB], BF16, isOutput=False)
    wq_d = nc.declare_dram_parameter("wq", [128, MC * FKC * 128], BF16, isOutput=False)
    wk_d = nc.declare_dram_parameter("wk", [128, MC * FKC * 128], BF16, isOutput=False)
    wv_d = nc.declare_dram_parameter("wv", [128, FKC * HPC * D], BF16, isOutput=False)
    bqk_d = nc.declare_dram_parameter("bqk", [128, 2 * MC], F32, isOutput=False)
    bvb_d = nc.declare_dram_parameter("bvb", [128, HPC * D], F32, isOutput=False)
    msk_d = nc.declare_dram_parameter("msk", [128, n_bnd * QB], BF16, isOutput=False)
    # output: ctx in [q_part, (qchunk, head, d)] layout, 16 chunks of 128 q
    out_d = nc.declare_dram_parameter("ctx", [128, 4 * NQB * HPC * D], F32, isOutput=True)

    with ExitStack() as ctx:
        tc = ctx.enter_context(tile.TileContext(nc))
        persist = ctx.enter_context(tc.tile_pool(name="persist", bufs=1))

        qt = persist.tile([128, MC * S], BF16)
        kt = persist.tile([128, MC * S], BF16)
        v = persist.tile([128, NKC * VW], BF16)
        ctxq = persist.tile([128, 4 * NQB * HPC * D], F32)
        msk = persist.tile([128, n_bnd * QB], BF16)
        bqk_sb = persist.tile([128, 2 * MC], F32)
        bv_sb = persist.tile([128, HPC * D], F32)
        xt = persist.tile([128, NB * FKC * QB], BF16)
        wq_sb = persist.tile([128, MC * FKC * 128], BF16)
        wk_sb = persist.tile([128, MC * FKC * 128], BF16)
        wv_sb = persist.tile([128, FKC * HPC * D], BF16)

        # DMAs, ordered so the first projection tiles unblock earliest.
        XB = FKC * QB  # xt columns per S-chunk
        WB = FKC * 128  # wq/wk columns per mc chunk
        nc.sync.dma_start(out=wq_sb[:, 0:WB], in_=wq_d[:, 0:WB])
        TB = 2 * QB  # first-chunk piece: 2 feature chunks
        for t in range(3):
            nc.sync.dma_start(out=xt[:, t * TB : (t + 1) * TB], in_=xt_d[:, t * TB : (t + 1) * TB])
        nc.sync.dma_start(out=wk_sb[:, 0:WB], in_=wk_d[:, 0:WB])
        nc.sync.dma_start(out=bqk_sb, in_=bqk_d[:])
        nc.sync.dma_start(out=bv_sb, in_=bvb_d[:])
        nc.sync.dma_start(out=wv_sb, in_=wv_d[:])
        for nb in range(1, NB):
            for t in range(3):
                c0 = nb * XB + t * TB
                nc.sync.dma_start(out=xt[:, c0 : c0 + TB], in_=xt_d[:, c0 : c0 + TB])
        for mc in range(1, MC):
            nc.sync.dma_start(
                out=wq_sb[:, mc * WB : (mc + 1) * WB], in_=wq_d[:, mc * WB : (mc + 1) * WB]
            )
            nc.sync.dma_start(
                out=wk_sb[:, mc * WB : (mc + 1) * WB], in_=wk_d[:, mc * WB : (mc + 1) * WB]
            )
        nc.sync.dma_start(out=msk, in_=msk_d[:])

        # ones columns of v_aug (overwritten nowhere below)
        v_ones = v.rearrange("p (s h e) -> p s h e", h=HPC, e=EW)[:, :, :, D : D + 1]
        nc.vector.memset(v_ones, 1.0)

        def xt_ap(nb, fkc, c0, c1):
            base = nb * XB + fkc * QB
            return xt[:, base + c0 : base + c1]

        def proj_qk(pool, pi, mc, nb):
            w_sb, out_sb = (wq_sb, qt) if pi == 0 else (wk_sb, kt)
            ps = pool.tile([128, QB], F32, tag="fp")
            for fkc in range(FKC):
                nc.tensor.matmul(
                    ps,
                    lhsT=w_sb[:, mc * WB + fkc * 128 : mc * WB + (fkc + 1) * 128],
                    rhs=xt_ap(nb, fkc, 0, QB),
                    start=(fkc == 0),
                    stop=(fkc == FKC - 1),
                )
            nc.vector.tensor_scalar_add(
                out_sb[:, mc * S + nb * QB : mc * S + (nb + 1) * QB],
                ps,
                bqk_sb[:, pi * MC + mc : pi * MC + mc + 1],
            )

        # ---- Phase A: mc0 q/k projections + all of V (PE-dense) ----
        with tc.tile_pool(name="fatproj", bufs=3, space="PSUM") as fat:
            for nb in range(NB):
                proj_qk(fat, 0, 0, nb)
                proj_qk(fat, 1, 0, nb)
            for sc in range(NKC):
                nb, i = sc // 4, sc % 4
                ps = fat.tile([128, HPC * D], F32, tag="fp")
                for fkc in range(FKC):
                    nc.tensor.matmul(
                        ps,
                        lhsT=xt_ap(nb, fkc, i * 128, (i + 1) * 128),
                        rhs=wv_sb[:, fkc * (HPC * D) : (fkc + 1) * (HPC * D)],
                        start=(fkc == 0),
                        stop=(fkc == FKC - 1),
                    )
                dest = v.rearrange("p (s h e) -> p s h e", h=HPC, e=EW)[:, sc, :, 0:D]
                nc.vector.tensor_add(
                    dest,
                    ps.rearrange("p (h e) -> p h e", e=D),
                    bv_sb.rearrange("p (h e) -> p h e", e=D),
                )
            for mc in range(1, MC):
                for nb in range(NB):
                    proj_qk(fat, 1, mc, nb)
                    proj_qk(fat, 0, mc, nb)

        # ---- Phase B: attention; mc1/mc2 projections between pairs ----
        with (
            tc.tile_pool(name="expp", bufs=3) as expp,
            tc.tile_pool(name="drainp", bufs=2) as drainp,
        ):
            mask_rr = 0  # round-robin DVE/GpSimd router for mask muls
            with (
                tc.tile_pool(name="scpsA", bufs=1, space="PSUM") as scps,
                tc.tile_pool(name="ctxpsA", bufs=1, space="PSUM") as ctxps,
            ):
                for hp in range(MC):
                    for qb in range(NQB):
                        vis = vis_lists[qb]
                        groups = []
                        cur, curw = [], 0
                        for kc in vis:
                            w = QB - qos.get((kc, qb), 0) * KC
                            if curw + w > G * QB and cur:
                                groups.append(cur)
                                cur, curw = [], 0
                            cur.append(kc)
                            curw += w
                        if cur:
                            groups.append(cur)
                        cps = {
                            par: ctxps.tile([128, 512], F32, tag=f"c{par}", name=f"cps{par}")
                            for par in range(2)
                        }
                        first = True
                        n_done = 0
                        deferred = None  # (g, esb) of the previous group
                        for g in groups:
                            # pack only the visible q-subrange of each kc
                            pk = []
                            off = 0
                            for kc in g:
                                qo = qos.get((kc, qb), 0)
                                w = QB - qo * KC
                                pk.append((kc, off, qo, w))
                                off += w
                            n = off
                            sps = {}
                            esb = {}
                            for par in range(2):
                                sps[par] = scps.tile(
                                    [128, G * QB], F32, tag=f"s{par}", name=f"sps{par}"
                                )
                                esb[par] = expp.tile(
                                    [128, G * QB], BF16, tag=f"e{par}", name=f"esb{par}"
                                )
                            for kc, joff, qo, w in pk:
                                for par in range(2):
                                    po = par * 64
                                    nc.tensor.matmul(
                                        sps[par][:, joff : joff + w],
                                        lhsT=kt[po : po + 64, hp * S + kc * KC : hp * S + kc * KC + KC],
                                        rhs=qt[po : po + 64, hp * S + qb * QB + qo * KC : hp * S + (qb + 1) * QB],
                                        start=True,
                                        stop=True,
                                    )
                            for par in range(2):
                                nc.scalar.activation(
                                    out=esb[par][:, :n],
                                    in_=sps[par][:, :n],
                                    func=mybir.ActivationFunctionType.Exp,
                                    scale=0.125,
                                )
                            # boundary masks (0/1, post-exp), 2:1 DVE:GpSimd
                            for kc, joff, qo, w in pk:
                                bi = bnd_index.get((kc, qb))
                                if bi is not None:
                                    for par in range(2):
                                        eng = nc.gpsimd if mask_rr % 3 == 0 else nc.vector
                                        mask_rr += 1
                                        eng.tensor_mul(
                                            esb[par][:, joff : joff + w],
                                            esb[par][:, joff : joff + w],
                                            msk[:, bi * QB + qo * KC : (bi + 1) * QB],
                                        )
                            # flipped ctx (esb stationary, v_aug moving) is
                            # emitted ONE GROUP LATE so the next group's
                            # scores matmuls sit ahead of ACT-dependent work
                            # in the PE queue (else ACT gates its own input).
                            n_done += len(g)
                            last_g = n_done == len(vis)

                            def emit_ctx(cpk, cesb, is_last):
                                nonlocal first
                                for idx, (kc, joff, qo, w) in enumerate(cpk):
                                    last_kc = is_last and (idx == len(cpk) - 1)
                                    for par in range(2):
                                        hg = 2 * hp + par
                                        for chk in range(qo, 4):
                                            nc.tensor.matmul(
                                                cps[par][:, chk * EW : (chk + 1) * EW],
                                                lhsT=cesb[par][:, joff + (chk - qo) * 128 : joff + (chk - qo + 1) * 128],
                                                rhs=v[:, kc * VW + hg * EW : kc * VW + (hg + 1) * EW],
                                                start=(first and chk == 0),
                                                stop=(last_kc and chk == 3),
                                                skip_group_check=True,
                                            )
                                        if par == 1:
                                            first = False

                            if deferred is not None:
                                emit_ctx(deferred[0], deferred[1], False)
                            deferred = (pk, esb)
                            # PE keep-warm: the HAM re-throttles the PE to
                            # 1.2 GHz unless its activity window stays
                            # saturated, so pad each group with dependency-
                            # free dummy matmuls (v read-only, dead upper
                            # half of the ctx banks, start=False, never read).
                            nd = 4 if not last_g else 2
                            for di in range(nd):
                                for par in range(2):
                                    nc.tensor.matmul(
                                        cps[par][:, 260:512],
                                        lhsT=v[:, 0:128],
                                        rhs=v[:, 0:252],
                                        start=False,
                                        stop=False,
                                        skip_group_check=True,
                                    )
                        emit_ctx(deferred[0], deferred[1], True)
                        # drain: per head, copy bank out, recip the 4 denom
                        # cols, one broadcast mul into the [q,d] output tile
                        for par in range(2):
                            hg = 2 * hp + par
                            cb = drainp.tile([128, 4 * EW], F32, tag=f"cb{par}", name=f"cb{par}")
                            nc.vector.tensor_copy(cb, cps[par][:, : 4 * EW])
                            cb_v = cb.rearrange("p (c e) -> p c e", e=EW)
                            rc = drainp.tile([128, 4], F32, tag=f"rc{par}", name=f"rc{par}")
                            nc.vector.tensor_copy(rc, cb_v[:, :, D : D + 1])
                            ri = drainp.tile([128, 4], F32, tag=f"ri{par}", name=f"ri{par}")
                            nc.vector.reciprocal_approx_fast(out=ri, in_=rc)
                            dst = ctxq.rearrange("p (c h e) -> p c h e", h=HPC, e=D)[
                                :, 4 * qb : 4 * qb + 4, hg, :
                            ]
                            nc.vector.tensor_tensor(
                                out=dst,
                                in0=cb_v[:, :, 0:D],
                                in1=ri.rearrange("p (c o) -> p c o", o=1).broadcast_to((128, 4, D)),
                                op=mybir.AluOpType.mult,
                            )
                        if hp == MC - 1:
                            cpq = 4 * HPC * D  # output columns per qb
                            nc.sync.dma_start(
                                out=out_d[:, qb * cpq : (qb + 1) * cpq],
                                in_=ctxq[:, qb * cpq : (qb + 1) * cpq],
                            )

    nc.finalize()
    return nc


def _core_inputs(x, segment_ids, Wq, bq, Wk, bk, Wv, bv, cs, bnd_index, core):
    b, h0 = core // 2, HPC * (core % 2)
    cols = slice(h0 * D, (h0 + HPC) * D)
    xT = np.ascontiguousarray(x[b].T)  # [768, 2048]
    # [128, nb, fkc, 512] layout: S-major chunks, feature-chunk minor
    xt_s = (
        xT.reshape(FKC, 128, NB, QB).transpose(1, 2, 0, 3).reshape(128, NB * FKC * QB)
    ).astype(ml_dtypes.bfloat16)

    def wqk_prep(Wm):
        ws = Wm[:, cols]  # [768, 384]
        return np.ascontiguousarray(
            ws.reshape(FKC, 128, MC, 128).transpose(1, 2, 0, 3).reshape(128, MC * FKC * 128)
        ).astype(ml_dtypes.bfloat16)

    ws = Wv[:, cols]
    wv_s = np.ascontiguousarray(
        ws.reshape(FKC, 128, HPC * D).transpose(1, 0, 2).reshape(128, FKC * HPC * D)
    ).astype(ml_dtypes.bfloat16)

    bq_s = np.ascontiguousarray(bq[cols].reshape(MC, 128).T)
    bk_s = np.ascontiguousarray(bk[cols].reshape(MC, 128).T)
    bqk = np.concatenate([bq_s, bk_s], axis=1)  # [128, 6]
    bvb = np.ascontiguousarray(np.broadcast_to(bv[cols], (128, HPC * D)))

    csb = cs[b]
    n_bnd = max(len(bnd_index), 1)
    mskv = np.zeros((128, n_bnd * QB), np.float32)
    for (kc, qb), bi in bnd_index.items():
        mskv[:, bi * QB : (bi + 1) * QB] = (
            csb[kc * KC : (kc + 1) * KC, None] <= csb[None, qb * QB : (qb + 1) * QB]
        )
    return {
        "xT": np.ascontiguousarray(xt_s),
        "wq": wqk_prep(Wq),
        "wk": wqk_prep(Wk),
        "wv": wv_s,
        "bqk": np.ascontiguousarray(bqk),
        "bvb": bvb,
        "msk": mskv.astype(ml_dtypes.bfloat16),
    }


def kernel(x, segment_ids, Wq, bq, Wk, bk, Wv, bv):
    global LAST_RESULTS
    x = np.asarray(x, np.float32)
    segment_ids = np.asarray(segment_ids)
    Wq, bq = np.asarray(Wq, np.float32), np.asarray(bq, np.float32)
    Wk, bk = np.asarray(Wk, np.float32), np.asarray(bk, np.float32)
    Wv, bv = np.asarray(Wv, np.float32), np.asarray(bv, np.float32)

    cs, vis_lists, bnd_index, qos = _classify(segment_ids)
    nc = _build_program(vis_lists, bnd_index, qos)
    in_maps = [
        _core_inputs(x, segment_ids, Wq, bq, Wk, bk, Wv, bv, cs, bnd_index, c)
        for c in range(NCORES)
    ]
    if TRACE:
        _ensure_ntff_hook()
    res = run_bass_kernel_spmd(nc, in_maps, list(range(NCORES)), trace=TRACE)
    LAST_RESULTS = res

    out = np.empty((B, S, W), np.float32)
    for c in range(NCORES):
        b, h0 = c // 2, HPC * (c % 2)
        # [128, 16, 384] -> [16, 128, 384] -> [2048, 384]
        cq = res.results[c]["ctx"].reshape(128, 4 * NQB, HPC * D)
        out[b, :, h0 * D : (h0 + HPC) * D] = cq.transpose(1, 0, 2).reshape(S, HPC * D)
    return out
